# revision 23
# baseline (speedup 1.0000x reference)
"""HGT (heterogeneous graph transformer) Bass kernel for Trainium2, 8 NeuronCores.

Strategy (dst-sharded edges):
  - Destination nodes (per type) are sharded over the 8 cores; softmax +
    segment-sum are core-local (no all-reduce).
  - Node projections are node-sharded; (k_r|v_r) tables are AllGathered (bf16).
  - Per-edge k_r/v_r rows come from one 128-row indirect DMA per 128-edge tile
    (the SWDGE descriptor-emission rate is the primary bottleneck lane).
  - q[dst] is expanded on the TensorEngine with a one-hot matmul (edges sorted
    by dst => q rows of a window are SBUF-local); segment softmax/sum are
    one-hot matmuls accumulating in PSUM per 128-dst window; max-subtraction
    is skipped (logits are O(0.1)).
  - a_rel/m_rel/p_rel/scale folded into effective weights on the host.
  - Layers alternate relation order so the next layer's first AllGather (and
    the dense out/node passes feeding it) overlap the second relation's edge
    phase; gelu runs bulk in-place on g_fm so the ACT table isn't thrashed
    between Exp and Gelu.
"""
import os
import sys

import numpy as np

try:
    import concourse  # noqa: F401
except ImportError:  # pragma: no cover
    sys.path.insert(0, "/opt/trn_rl_repo")

import ml_dtypes

import concourse.bacc as bacc
import concourse.bass as bass
import concourse.tile as tile
from concourse import mybir
from concourse.bass_utils import run_bass_kernel_spmd

f32 = mybir.dt.float32
bf16 = mybir.dt.bfloat16
i32 = mybir.dt.int32
f8 = mybir.dt.float8e4
AF = mybir.ActivationFunctionType
ALU = mybir.AluOpType
BF = ml_dtypes.bfloat16

FULL_CFG = dict(N=100000, E=500000, HID=128, H=4, D=32, L=2, NC=8)
USE_TTR = os.environ.get('TTR', '0') == '1'  # hangs trn2 HW in this env  # fuse qk-mul+per-head-reduce into tensor_tensor_reduce


def _blockdiag(a):  # a: [H, D, D] -> [H*D, H*D]
    H, D, _ = a.shape
    out = np.zeros((H * D, H * D), np.float32)
    for h in range(H):
        out[h * D:(h + 1) * D, h * D:(h + 1) * D] = a[h]
    return out


def host_prep(inputs, cfg):
    N, E, HID, H, D, L, NC = (cfg[k] for k in ("N", "E", "HID", "H", "D", "L", "NC"))
    NSH = N // NC
    W = (NSH + 127) // 128
    NP = W * 128

    ip = {k: np.asarray(v) for k, v in inputs.items()}
    rel_st = [0, 1]
    rel_dt = [1, 0]
    edges = [ip["edge_ui"], ip["edge_iu"]]

    # ---- effective weights ----
    scale = 1.0 / np.sqrt(D)
    W3 = np.zeros((L, 2, HID, 3 * HID), np.float32)  # (l, r): [Wk_eff|Wv_eff|Wq_eff(t=r)]
    b3 = np.zeros((L, 2, 3 * HID), np.float32)
    for l in range(L):
        for r in range(2):
            st, dt = rel_st[r], rel_dt[r]
            BDa = _blockdiag(ip["a_rel"][l, r])
            BDm = _blockdiag(ip["m_rel"][l, r])
            W3[l, r, :, 0:HID] = ip["Wk"][l, st] @ BDa
            b3[l, r, 0:HID] = ip["bk"][l, st] @ BDa
            W3[l, r, :, HID:2 * HID] = ip["Wv"][l, st] @ BDm
            b3[l, r, HID:2 * HID] = ip["bv"][l, st] @ BDm
        for t in range(2):
            r_of = 1 - t  # relation whose dst type is t
            pscale = np.repeat(ip["p_rel"][l, r_of] * scale, D)
            W3[l, t, :, 2 * HID:3 * HID] = ip["Wq"][l, t] * pscale[None, :]
            b3[l, t, 2 * HID:3 * HID] = ip["bq"][l, t] * pscale
    beta = 1.0 / (1.0 + np.exp(-ip["skip"]))  # [L, T]

    # ---- degree-balanced node -> (core, window) assignment per type ----
    # Type t is the dst of relation 1-t. Packing every (core, window) bin to
    # ~mean edge count (637.7 < 5*128) makes nearly every window exactly 5
    # tiles, minimizing the per-tile SWDGE gather instructions (the
    # bottleneck lane).
    import heapq

    NBINS = NC * W
    perm_g = []  # perm_g[t][node] = padded global slot (c*NP + w*128 + j)
    for t in range(2):
        deg = np.bincount(edges[1 - t][1].astype(np.int64), minlength=N)
        order = np.argsort(-deg, kind="stable")
        binfill = np.zeros(NBINS, np.int64)
        gof = np.empty(N, np.int64)
        heap = [(0, b) for b in range(NBINS)]
        heapq.heapify(heap)
        CAP = 5 * 128  # hard bin capacity => every window is exactly 5 tiles
        spill = []
        for n in order:
            dn = int(deg[n])
            while True:
                s, b = heapq.heappop(heap)
                if binfill[b] < 128:
                    break
            if dn > 0 and s + dn > CAP:
                # lightest bin can't take it without exceeding CAP
                spill.append(n)
                heapq.heappush(heap, (s, b))
                continue
            gof[n] = (b // W) * NP + (b % W) * 128 + binfill[b]
            binfill[b] += 1
            if binfill[b] < 128:
                heapq.heappush(heap, (s + dn, b))
        for n in spill:  # extremely unlikely; place wherever there is space
            b = int(np.argmax(binfill < 128))
            gof[n] = (b // W) * NP + (b % W) * 128 + binfill[b]
            binfill[b] += 1
        perm_g.append(gof)

    # ---- edge schedules (identical across cores) ----
    def prep_rel(e, st, dt):
        src, dst = e[0].astype(np.int64), e[1].astype(np.int64)
        gsrc = perm_g[st][src]
        gd = perm_g[dt][dst]
        gcore = gd // NP
        per_core = []
        counts = np.zeros((NC, W), np.int64)
        for c in range(NC):
            sel = gcore == c
            s_c = gsrc[sel]
            dl_c = gd[sel] - c * NP
            order = np.argsort(dl_c, kind="stable")
            s_c, dl_c = s_c[order], dl_c[order]
            counts[c] = np.bincount(dl_c // 128, minlength=W)
            per_core.append((s_c, dl_c))
        tiles_w = np.maximum(1, (counts.max(axis=0) + 127) // 128)
        NT = int(tiles_w.sum())
        idx_src = np.zeros((NC, NT * 128), np.int32)
        dloc = np.full((NC, NT * 128), 128.0, np.float32)  # 128 => pad slot
        for c in range(NC):
            s_c, dl_c = per_core[c]
            starts = np.concatenate([[0], np.cumsum(counts[c])])
            slot = 0
            for w in range(W):
                n = int(counts[c][w])
                a, b = int(starts[w]), int(starts[w]) + n
                idx_src[c, slot:slot + n] = s_c[a:b]
                dloc[c, slot:slot + n] = (dl_c[a:b] % 128).astype(np.float32)
                slot += int(tiles_w[w]) * 128
        return tiles_w, NT, idx_src, dloc

    schedules = []
    meta = []
    for r in range(2):
        tiles_w, NT, idx_src, dloc = prep_rel(edges[r], rel_st[r], rel_dt[r])
        schedules.append((tiles_w, NT))
        meta.append((idx_src, dloc))

    # ---- per-core input arrays ----
    NTtot = schedules[0][1] + schedules[1][1]
    xs = [ip["x_user"].astype(np.float32), ip["x_item"].astype(np.float32)]
    in_maps = []
    gcore_t = [perm_g[t] // NP for t in range(2)]
    gloc_t = [perm_g[t] % NP for t in range(2)]
    for c in range(NC):
        x_fm = np.zeros((2, HID, NP), BF)
        for t in range(2):
            m = gcore_t[t] == c
            x_fm[t, :, gloc_t[t][m]] = xs[t][m].astype(BF)
        idx_cat = np.concatenate(
            [meta[0][0][c], meta[1][0][c]]).reshape(NTtot, 128).T
        dl = np.concatenate([meta[0][1][c], meta[1][1][c]])
        dloc_col = dl.reshape(NTtot, 128).T.astype(np.float32).copy()
        in_maps.append({
            "x_fm": x_fm,
            "idx_src": np.ascontiguousarray(idx_cat.astype(np.int32)),
            "dloc_col": np.ascontiguousarray(dloc_col),
        })

    # bias cols [128, NB] f32: 0,1 = b_in; 2.. = beta*bo (l,t)
    b_list = [ip["b_in"][0], ip["b_in"][1]]
    for l in range(L):
        for t in range(2):
            b_list.append(beta[l, t] * ip["bo"][l, t])
    Bcols = np.stack(b_list).astype(np.float32)

    bias_nz = [[bool(np.any(b3[l, r] != 0)) for r in range(2)] for l in range(L)]
    consts = {
        "bias_nz": bias_nz,
        "iota_row": np.tile(np.arange(128, dtype=np.float32), (128, 1)).astype(BF),
        "ident": np.eye(128, dtype=np.float32).astype(BF),
        "W3": W3.reshape(L * 2, HID, 3 * HID).astype(BF),
        "Win": ip["W_in"].astype(np.float32).astype(BF),
        "Wo_bf": ip["Wo"].astype(np.float32).reshape(L * 2, HID, HID).astype(BF),
        "b3": b3.reshape(1, L * 2 * 3 * HID).astype(np.float32),
        "Bcols": Bcols,
        "ones1f": np.ones((1, 128), np.float32),
    }
    dims = dict(NSH=NSH, W=W, NP=NP, NTtot=NTtot,
                gcore=gcore_t, gloc=gloc_t)
    return in_maps, consts, bases_dummy(), schedules, dims, beta


def bases_dummy():
    return {}


def build_program(cfg, consts, bases, schedules, dims, beta, sim_gelu=False):
    N, E, HID, H, D, L, NC = (cfg[k] for k in ("N", "E", "HID", "H", "D", "L", "NC"))
    NSH, W, NP, NTtot = dims["NSH"], dims["W"], dims["NP"], dims["NTtot"]
    NPALL = NP * NC
    rel_dt = [1, 0]
    NB = consts["Bcols"].shape[0]
    CHD = 448 if NP % 448 == 0 else 128      # dense (psum) chunk width
    CHN = 896 if NP % 896 == 0 else (512 if NP % 512 == 0 else NP)  # node h chunk
    assert NP % CHD == 0 and NP % CHN == 0 and CHN % 128 == 0

    nc = bacc.Bacc("TRN2", target_bir_lowering=False, debug=False, num_devices=NC,
                   num_swdge_queues=4)

    x_fm = nc.dram_tensor("x_fm", [2, HID, NP], bf16, kind="ExternalInput")
    idx_src = nc.dram_tensor("idx_src", [128, NTtot], i32, kind="ExternalInput")
    dloc_col_d = nc.dram_tensor("dloc_col", [128, NTtot], f32, kind="ExternalInput")
    it_row_d = nc.dram_tensor("iota_row", [128, 128], bf16, kind="ExternalInput")
    ident_d = nc.dram_tensor("ident", [128, 128], bf16, kind="ExternalInput")
    W3_d = nc.dram_tensor("W3", [L * 2, HID, 3 * HID], bf16, kind="ExternalInput")
    Win_d = nc.dram_tensor("Win", [2, HID, HID], bf16, kind="ExternalInput")
    Wo_d = nc.dram_tensor("Wo_bf", [L * 2, HID, HID], bf16, kind="ExternalInput")
    b3_d = nc.dram_tensor("b3", [1, L * 2 * 3 * HID], f32, kind="ExternalInput")
    Bcols_d = nc.dram_tensor("Bcols", [NB, HID], f32, kind="ExternalInput")
    ones1f_d = nc.dram_tensor("ones1f", [1, 128], f32, kind="ExternalInput")
    out_d = nc.dram_tensor("out", [2, HID, NP], f32, kind="ExternalOutput")

    with tile.TileContext(nc) as tc:
        with tc.tile_pool(name="persist", bufs=1) as pp, \
             tc.tile_pool(name="dram", bufs=1, space="DRAM") as dp, \
             tc.tile_pool(name="wk_sb", bufs=3) as sb3, \
             tc.tile_pool(name="wk_sb2", bufs=2) as sb2, \
             tc.tile_pool(name="gath", bufs=16) as gwin, \
             tc.tile_pool(name="edge8", bufs=10) as sb8, \
             tc.tile_pool(name="ps_edge", bufs=2, space="PSUM") as ps_e, \
             tc.tile_pool(name="ps_dense", bufs=2, space="PSUM") as ps_d:

            # --- persistent SBUF ---
            it_row = pp.tile([128, 128], bf16)
            nc.sync.dma_start(it_row[:], it_row_d[:])
            ident = pp.tile([128, 128], bf16)
            nc.sync.dma_start(ident[:], ident_d[:])
            onesf = pp.tile([1, 128], f32)
            nc.sync.dma_start(onesf[:], ones1f_d[:])
            idxs = pp.tile([128, NTtot], i32)
            nc.sync.dma_start(idxs[:], idx_src[:])
            dloc_col = pp.tile([128, NTtot], f32)
            nc.sync.dma_start(dloc_col[:], dloc_col_d[:])
            w3sb = pp.tile([128, L * 2, 3 * HID], bf16)
            nc.sync.dma_start(w3sb[:], W3_d[:].rearrange("k p d -> p k d"))
            winsb = pp.tile([128, 2, HID], bf16)
            nc.sync.dma_start(winsb[:], Win_d[:].rearrange("k p d -> p k d"))
            wosb = pp.tile([128, L * 2, HID], bf16)
            nc.sync.dma_start(wosb[:], Wo_d[:].rearrange("k p d -> p k d"))
            b3sb = pp.tile([1, L * 2 * 3 * HID], f32)
            nc.sync.dma_start(b3sb[:], b3_d[:])
            bcols = pp.tile([128, NB], f32)
            nc.sync.dma_start(bcols[:], Bcols_d[:].rearrange("k d -> d k"))

            q_sb = [pp.tile([128, W, 128], bf16, name=f"q_sb{t}") for t in range(2)]
            g_fm = [pp.tile([128, NP], bf16, name=f"g_fm{t}") for t in range(2)]

            hA = [dp.tile([128, NP], bf16, name=f"hA{t}") for t in range(2)]
            hB = [dp.tile([128, NP], bf16, name=f"hB{t}") for t in range(2)]
            kvloc = [dp.tile([NP, 256], f8, name=f"kvloc{r}") for r in range(2)]
            kvfull = [[dp.tile([NPALL, 256], f8, name=f"kvfull{l}{r}")
                       for r in range(2)] for l in range(L)]
            rg = [list(range(NC))]

            def b3row(l, r, lo, hi):  # bias row slice [1, hi-lo]
                base = (l * 2 + r) * 3 * HID
                return b3sb[:, base + lo:base + hi]

            # dense projection pass over the node shard, writing kv and/or q
            def node_pass(l, r, h_src, do_kv, do_q):
                lo = 0 if do_kv else 2 * HID
                hi = 3 * HID if do_q else 2 * HID
                ncols = hi - lo
                for jc in range(NP // CHN):
                    hch = sb3.tile([128, CHN], bf16, tag="hch")
                    nc.sync.dma_start(hch[:], h_src[:, jc * CHN:(jc + 1) * CHN])
                    for k in range(CHN // 128):
                        w = jc * (CHN // 128) + k
                        ps = ps_d.tile([128, 3 * HID], f32, tag="dense")
                        bias_nz = consts["bias_nz"][l][r]
                        nc.tensor.matmul(
                            out=ps[:, :ncols], lhsT=hch[:, k * 128:(k + 1) * 128],
                            rhs=w3sb[:, l * 2 + r, lo:hi], start=True,
                            stop=not bias_nz)
                        if bias_nz:
                            nc.tensor.matmul(
                                out=ps[:, :ncols], lhsT=onesf[:],
                                rhs=b3row(l, r, lo, hi), start=False, stop=True)
                        if do_kv:
                            kvt = sb3.tile([128, 256], f8, tag="kvt")
                            nc.scalar.activation(kvt[:], ps[:, 0:256], AF.Copy)
                            nc.sync.dma_start(
                                kvloc[r][w * 128:(w + 1) * 128, :], kvt[:])
                        if do_q:
                            nc.vector.tensor_copy(
                                q_sb[r][:, w, :], ps[:, ncols - HID:ncols])

            def edge_phase(l, r, tbase):
                """3-stage software pipeline over windows: gather(w+2) /
                prep(w+1: S one-hots, transposes, St copy) / compute(w).
                Engine queues are in-order, so each engine always has the
                next window's independent work queued ahead of this
                window's cross-engine round trip."""
                tiles_w, NT = schedules[r]
                dt = rel_dt[r]
                NTM = int(tiles_w.max())
                tstart = np.concatenate([[0], np.cumsum(tiles_w)]) + tbase
                kvws = {}
                preps = {}
                qrot = [0]

                def stage_gather(w):
                    nt, t_idx = int(tiles_w[w]), int(tstart[w])
                    kvw = gwin.tile([128, NTM, 256], f8, tag="g")
                    for i in range(nt):
                        gi = nc.gpsimd.indirect_dma_start(
                            out=kvw[:, i, :], out_offset=None,
                            in_=kvfull[l][r][:],
                            in_offset=bass.IndirectOffsetOnAxis(
                                ap=idxs[:, t_idx + i:t_idx + i + 1], axis=0))
                        # rotate across the 4 SWDGE rings so one ring drains
                        # while the next desc-gen runs (each 128-desc gather
                        # fills a whole ring)
                        qn = qrot[0] % 4
                        gi.ins.queue = f"qPoolDynamic{qn if qn else ''}"
                        qrot[0] += 1
                    kvws[w] = kvw

                def stage_prep(w):
                    nt, t_idx = int(tiles_w[w]), int(tstart[w])
                    S2 = sb8.tile([128, NTM, 128], bf16, tag="S")
                    for i in range(nt):
                        nc.vector.tensor_scalar(
                            out=S2[:, i, :], in0=it_row[:],
                            scalar1=dloc_col[:, t_idx + i:t_idx + i + 1],
                            scalar2=None, op0=ALU.is_equal)
                    psst = ps_e.tile([128, NTM, 128], bf16, tag="st")
                    for i in range(nt):
                        nc.tensor.transpose(out=psst[:, i, :], in_=S2[:, i, :],
                                            identity=ident[:])
                    St2 = sb8.tile([128, NTM, 128], bf16, tag="St")
                    nc.scalar.activation(St2[:, 0:nt, :], psst[:, 0:nt, :],
                                         AF.Copy)
                    preps[w] = (S2, St2)

                def stage_compute(w):
                    nt = int(tiles_w[w])
                    kvw = kvws.pop(w)
                    S2, St2 = preps.pop(w)
                    pswin = ps_e.tile([128, 132], f32, tag="win")
                    pay = sb8.tile([128, NTM, 132], bf16, tag="pay")
                    lg = sb8.tile([128, NTM, 4], f32, tag="lg")
                    for (k0, G) in [(k, min(3, nt - k)) for k in range(0, nt, 3)]:
                        psqe = ps_e.tile([128, 3, 128], f32, tag="qe")
                        for i in range(G):
                            nc.tensor.matmul(out=psqe[:, i, :],
                                             lhsT=St2[:, k0 + i, :],
                                             rhs=q_sb[dt][:, w, :],
                                             start=True, stop=True)
                        qk = sb8.tile([128, 3, 128], bf16, tag="qk")
                        nc.vector.tensor_tensor(
                            out=qk[:, :G, :],
                            in0=psqe[:, :G, :],
                            in1=kvw[:, k0:k0 + G, 0:128],
                            op=ALU.mult)
                        nc.vector.tensor_reduce(
                            out=lg[:, k0:k0 + G, :],
                            in_=qk[:, :G, :].rearrange("p g (h d) -> p (g h) d", h=H),
                            axis=mybir.AxisListType.X, op=ALU.add)
                    nc.scalar.activation(pay[:, 0:nt, 128:132], lg[:, 0:nt, :],
                                         AF.Exp)
                    nc.vector.tensor_tensor(
                        out=pay[:, 0:nt, 0:128].rearrange("p g (h d) -> p g h d", h=H),
                        in0=kvw[:, 0:nt, 128:256].rearrange("p g (h d) -> p g h d", h=H),
                        in1=pay[:, 0:nt, 128:132].to_broadcast([128, nt, H, D]),
                        op=ALU.mult)
                    for i in range(nt):
                        nc.tensor.matmul(out=pswin[:], lhsT=S2[:, i, :],
                                         rhs=pay[:, i, :],
                                         start=(i == 0), stop=(i == nt - 1))
                    # window flush: normalize agg, transpose into g_fm
                    zrw = sb8.tile([128, 4], f32, tag="zrw")
                    nc.vector.tensor_scalar(out=zrw[:], in0=pswin[:, 128:132],
                                            scalar1=1e-16, scalar2=None, op0=ALU.add)
                    nc.vector.reciprocal(zrw[:], zrw[:])
                    gt = sb8.tile([128, 128], bf16, tag="gt")
                    nc.vector.tensor_tensor(
                        out=gt[:].rearrange("p (h d) -> p h d", h=H),
                        in0=pswin[:, 0:128].rearrange("p (h d) -> p h d", h=H),
                        in1=zrw[:].to_broadcast([128, H, D]),
                        op=ALU.mult)
                    psgt = ps_e.tile([128, 128], bf16, tag="st")
                    nc.tensor.transpose(out=psgt[:], in_=gt[:], identity=ident[:])
                    nc.scalar.activation(g_fm[dt][:, w * 128:(w + 1) * 128], psgt[:],
                                         AF.Copy)

                for w in range(W + 3):
                    if w < W:
                        stage_gather(w)
                    if 2 <= w <= W + 1:
                        stage_prep(w - 2)
                    if w >= 3:
                        stage_compute(w - 3)

            def bulk_gelu(t, lo, hi):
                if not sim_gelu:
                    nc.scalar.activation(g_fm[t][:, lo:hi], g_fm[t][:, lo:hi],
                                         AF.Gelu)
                else:
                    tmp = sb2.tile([128, NP], f32, tag="sgl")
                    g = g_fm[t][:, lo:hi]
                    tm = tmp[:, lo:hi]
                    nc.vector.tensor_tensor(out=tm, in0=g, in1=g, op=ALU.mult)
                    nc.vector.tensor_scalar(out=tm, in0=tm, scalar1=0.044715,
                                            scalar2=1.0, op0=ALU.mult, op1=ALU.add)
                    nc.vector.tensor_tensor(out=tm, in0=tm, in1=g, op=ALU.mult)
                    nc.scalar.activation(tm, tm, AF.Tanh, scale=0.7978845608028654)
                    nc.vector.tensor_scalar(out=tm, in0=tm, scalar1=1.0, scalar2=0.5,
                                            op0=ALU.add, op1=ALU.mult)
                    nc.vector.tensor_tensor(out=g, in0=tm, in1=g, op=ALU.mult)

            def out_phase(l, t, h_src, dst, dst_f32):
                bb = 2 + l * 2 + t
                coef = float((1.0 - beta[l, t]) + (1.0 if l > 0 else 0.0))
                bulk_gelu(t, 0, NP)
                for j in range(NP // CHD):
                    sl = slice(j * CHD, (j + 1) * CHD)
                    ps = ps_d.tile([128, CHD], f32, tag="dense")
                    nc.tensor.matmul(out=ps[:], lhsT=wosb[:, l * 2 + t, :],
                                     rhs=g_fm[t][:, sl], start=True, stop=True)
                    a1 = sb2.tile([128, CHD], f32, tag="a1")
                    nc.vector.tensor_scalar(
                        out=a1[:], in0=ps[:], scalar1=float(beta[l, t]),
                        scalar2=bcols[:, bb:bb + 1], op0=ALU.mult, op1=ALU.add)
                    hch = sb2.tile([128, CHD], bf16, tag="hcho")
                    nc.sync.dma_start(hch[:], h_src[:, sl])
                    hn = sb2.tile([128, CHD], f32 if dst_f32 else bf16,
                                  tag="hn" if dst_f32 else "hnb")
                    nc.vector.scalar_tensor_tensor(
                        out=hn[:], in0=hch[:], scalar=coef, in1=a1[:],
                        op0=ALU.mult, op1=ALU.add)
                    nc.sync.dma_start(dst[:, sl], hn[:])

            def input_proj(t, dst):
                for j in range(NP // CHD):
                    sl = slice(j * CHD, (j + 1) * CHD)
                    xt = sb2.tile([128, CHD], bf16, tag="xt")
                    nc.sync.dma_start(xt[:], x_fm[t, :, sl])
                    ps = ps_d.tile([128, CHD], f32, tag="dense")
                    nc.tensor.matmul(out=ps[:], lhsT=winsb[:, t, :], rhs=xt[:],
                                     start=True, stop=True)
                    ht = sb2.tile([128, CHD], bf16, tag="ht")
                    nc.scalar.activation(ht[:], ps[:], AF.Relu,
                                         bias=bcols[:, t:t + 1], scale=1.0)
                    nc.sync.dma_start(dst[:, sl], ht[:])

            def fused_input_kv(r, do_q):
                """Prologue fast path: x -> relu(h) kept in SBUF, kv (and q)
                projections taken directly from it. Avoids the hA DRAM
                round-trip between input_proj and node_pass so the layer-0
                AllGather input is ready after one pipelined sweep."""
                hi = 3 * HID if do_q else 2 * HID
                for j in range(NP // 256):
                    sl = slice(j * 256, (j + 1) * 256)
                    xt = sb2.tile([128, 256], bf16, tag="xt2")
                    nc.sync.dma_start(xt[:], x_fm[r, :, sl])
                    ps = ps_d.tile([128, 3 * HID], f32, tag="dense")
                    nc.tensor.matmul(out=ps[:, 0:256], lhsT=winsb[:, r, :],
                                     rhs=xt[:], start=True, stop=True)
                    ht = sb2.tile([128, 256], bf16, tag="ht2")
                    nc.scalar.activation(ht[:], ps[:, 0:256], AF.Relu,
                                         bias=bcols[:, r:r + 1], scale=1.0)
                    nc.sync.dma_start(hA[r][:, sl], ht[:])
                    for k in range(2):
                        w = j * 2 + k
                        ps2 = ps_d.tile([128, 3 * HID], f32, tag="dense")
                        nc.tensor.matmul(out=ps2[:, 0:hi],
                                         lhsT=ht[:, k * 128:(k + 1) * 128],
                                         rhs=w3sb[:, r, 0:hi],
                                         start=True, stop=True)
                        kvt = sb3.tile([128, 256], f8, tag="kvt")
                        nc.scalar.activation(kvt[:], ps2[:, 0:256], AF.Copy)
                        nc.sync.dma_start(
                            kvloc[r][w * 128:(w + 1) * 128, :], kvt[:])
                        if do_q:
                            nc.vector.tensor_copy(
                                q_sb[r][:, w, :], ps2[:, 2 * HID:3 * HID])

            # ---------------- schedule ----------------
            # layer l relation order alternates so the dense chain for the next
            # layer's first AG overlaps the current second edge phase.
            rorder = [[0, 1], [1, 0]][: L] if L <= 2 else None
            if L > 2:
                rorder = [[0, 1] if l % 2 == 0 else [1, 0] for l in range(L)]
            tb = [0, schedules[0][1]]  # tile base per relation

            h_cur = hA
            # layer 0 dense, interleaved with input projections
            rF, rS = rorder[0]
            fused_input_kv(rF, do_q=False)
            nc.gpsimd.collective_compute("AllGather", ALU.bypass, replica_groups=rg,
                                         ins=[kvloc[rF][:]], outs=[kvfull[0][rF][:]])
            fused_input_kv(rS, do_q=True)
            nc.gpsimd.collective_compute("AllGather", ALU.bypass, replica_groups=rg,
                                         ins=[kvloc[rS][:]], outs=[kvfull[0][rS][:]])
            node_pass(0, rF, hA[rF], do_kv=False, do_q=True)

            for l in range(L):
                rF, rS = rorder[l]
                last = l == L - 1
                h_nxt = hB if l == 0 else None
                # edge rF -> g_fm[dt(rF)]
                edge_phase(l, rF, tb[rF])
                # dense chain that only depends on edge rF:
                tF_out = rel_dt[rF]
                dstF = (h_nxt[tF_out] if not last else out_d[tF_out])
                out_phase(l, tF_out, h_cur[tF_out], dstF, last)
                if not last:
                    l2 = l + 1
                    rF2, rS2 = rorder[l2]
                    # node passes for next layer that depend only on h_nxt[tF_out]
                    # rF2 == tF_out by construction of alternating order
                    node_pass(l2, rF2, h_nxt[rF2], do_kv=True, do_q=False)
                    nc.gpsimd.collective_compute(
                        "AllGather", ALU.bypass, replica_groups=rg,
                        ins=[kvloc[rF2][:]], outs=[kvfull[l2][rF2][:]])
                # edge rS
                edge_phase(l, rS, tb[rS])
                tS_out = rel_dt[rS]
                dstS = (h_nxt[tS_out] if not last else out_d[tS_out])
                out_phase(l, tS_out, h_cur[tS_out], dstS, last)
                if not last:
                    node_pass(l2, rS2, h_nxt[rS2], do_kv=True, do_q=True)
                    nc.gpsimd.collective_compute(
                        "AllGather", ALU.bypass, replica_groups=rg,
                        ins=[kvloc[rS2][:]], outs=[kvfull[l2][rS2][:]])
                    node_pass(l2, rF2, h_nxt[rF2], do_kv=False, do_q=True)
                    h_cur = hB

    nc.finalize()
    return nc


def run(inputs, cfg=None, trace=False, trace_cores=None, sim=False):
    cfg = cfg or FULL_CFG
    NC = cfg["NC"]
    core_maps, consts, bases, schedules, dims, beta = host_prep(inputs, cfg)
    nc = build_program(cfg, consts, bases, schedules, dims, beta, sim_gelu=sim)
    in_maps = []
    for c in range(NC):
        m = dict(core_maps[c])
        for k in ("iota_row", "ident", "W3", "Win", "Wo_bf", "b3", "Bcols",
                  "ones1f"):
            m[k] = consts[k]
        in_maps.append(m)
    if sim:
        from concourse.bass_interp import MultiCoreSim

        msim = MultiCoreSim(nc, num_cores=NC, trace=False,
                            require_finite=False, require_nnan=False)
        cores = [msim.cores[c] for c in range(NC)]
        for c in range(NC):
            for name, arr in in_maps[c].items():
                cores[c].tensor(name)[:] = arr
        msim.simulate(check_with_hw=False)

        class R:
            exec_time_ns = None
            results = [{"out": np.asarray(cores[c].tensor("out"))}
                       for c in range(NC)]
        res = R()
    else:
        res = run_bass_kernel_spmd(nc, in_maps, core_ids=list(range(NC)),
                                   trace=trace, trace_cores=trace_cores)
    NP = dims["NP"]
    out = np.empty((2, cfg["N"], cfg["HID"]), np.float32)
    for c in range(NC):
        o = res.results[c]["out"]
        for t in range(2):
            m = dims["gcore"][t] == c
            out[t, m] = o[t][:, dims["gloc"][t][m]].T
    return out, res


def kernel(**inputs):
    out, _ = run(inputs, FULL_CFG, trace=False)
    return out

